# revision 10
# baseline (speedup 1.0000x reference)
"""TRN2 Bass/Tile kernel for nn_MHA_45964740002076.

MHA: x[1,4096,768] -> qkv proj -> 12-head attention (softmax scaled by
1/sqrt(768) AFTER softmax, per reference) -> out proj.

Sharding (8 NeuronCores, SPMD, no collectives):
  - Sequence-parallel queries: core c owns q rows [c*512, (c+1)*512).
  - K/V are computed for the FULL sequence on every core (replicated
    compute; cheaper than an all-gather here and removes collective risk).
  - Each core writes its own transposed output block [768, 512]; the host
    transposes + concatenates.

Host-side prep (free): permute Wqkv into head-major Q/K/V blocks, fold
1/sqrt(768) into Wv/bv, transpose x, cast matmul inputs to bf16.

On-core pipeline (all matmul inputs bf16, fp32 PSUM accumulation):
  QT proj:  QT[pair,:]   = Wq^T xT_own   (pair = 2 heads = 128 rows)
  group passes (g=0..2, 2 head-pairs each): stream xT chunks from DRAM,
    KT[pair] = Wk^T xT (transposed layout), V[:, group cols] = x Wv
  attention per pair, 2 heads row-tiled on the PE (dh=64 contraction):
    scoresT[l,q] = KT_h^T-slice @ QT_h       (PSUM, fp32)
    expT = exp(scoresT)                      (ACT, one pass, no max-sub:
                                              |energy| < ~30, fp32-safe)
    out_aug[v,q] += V_aug[lt,h]^T @ expT     (V_aug has a ones column ->
                                              row 64 = softmax denominator)
    attnT_h = out_aug[0:64] * (1/out_aug[64]) + bv  (recip on DVE, bcast
              via tiny PE matmul into unused partitions 64:128 of the
              same PSUM bank)
  o-proj:  outT[o,n] = Wo^T attnT + bo  (bias via per-partition DVE add)
"""

import os
import numpy as np

os.environ.setdefault("MYCRO_LOCAL_CACHE", "1")

D = 768
H = 12
DH = 64
N = 4096
NCORES = 8
NLOC = N // NCORES          # 512 q rows per core
PAIRS = H // 2              # 6
GROUPS = 3                  # 2 pairs (4 heads) per group
ITILES = D // 128           # 6
NSLICES = N // 512          # 8
LTILES = N // 128           # 32

_cache = {}


def _build_program():
    import concourse.bass as bass
    import concourse.mybir as mybir
    import concourse.tile as tile
    from concourse import bacc

    f32 = mybir.dt.float32
    bf16 = mybir.dt.bfloat16
    mult = mybir.AluOpType.mult

    nc = bacc.Bacc("TRN2", target_bir_lowering=False, debug=False)

    xT = nc.dram_tensor("xT", [D, N], bf16, kind="ExternalInput").ap()
    xTo = nc.dram_tensor("xTo", [D, NLOC], bf16, kind="ExternalInput").ap()
    Wq = nc.dram_tensor("Wq", [D, D], bf16, kind="ExternalInput").ap()
    Wk = nc.dram_tensor("Wk", [D, D], bf16, kind="ExternalInput").ap()
    Wv = nc.dram_tensor("Wv", [D, D], bf16, kind="ExternalInput").ap()
    Wo = nc.dram_tensor("Wo", [D, D], bf16, kind="ExternalInput").ap()
    bq = nc.dram_tensor("bq", [D], f32, kind="ExternalInput").ap()
    bk = nc.dram_tensor("bk", [D], f32, kind="ExternalInput").ap()
    bv = nc.dram_tensor("bv", [D], f32, kind="ExternalInput").ap()
    bo = nc.dram_tensor("bo", [D], f32, kind="ExternalInput").ap()
    outT = nc.dram_tensor("outT", [D, NLOC], f32, kind="ExternalOutput").ap()

    with tile.TileContext(nc) as tc:
        with (
            tc.tile_pool(name="wpool", bufs=18) as wpool,
            tc.tile_pool(name="persist", bufs=1) as persist,
            tc.tile_pool(name="chunks", bufs=12) as chunks,
            tc.tile_pool(name="expp", bufs=3) as expp,
            tc.tile_pool(name="small", bufs=2) as small,
            tc.tile_pool(name="gp_ps", bufs=2, space=bass.MemorySpace.PSUM) as gp_ps,
            tc.tile_pool(name="sc_ps", bufs=2, space=bass.MemorySpace.PSUM) as sc_ps,
            tc.tile_pool(name="acc_ps", bufs=2, space=bass.MemorySpace.PSUM) as acc_ps,
        ):
            # ---- persistent SBUF state ----
            bias_t = {}
            for nm, dram in (("bq", bq), ("bk", bk), ("bv", bv), ("bo", bo)):
                t = persist.tile([128, ITILES], f32, tag=f"bias_{nm}", name=f"bias_{nm}")
                nc.sync.dma_start(t[:], dram.rearrange("(t p) -> p t", p=128))
                bias_t[nm] = t

            # ones row [1,64]: lhsT of the recip-broadcast matmul (K=1, M=64)
            ones_row = persist.tile([1, 64], bf16, tag="ones")
            nc.vector.memset(ones_row[:], 1.0)
            # explicit zero bias for ACT exp (per-partition [128,1])
            zbias = persist.tile([128, 1], f32, tag="zbias")
            nc.vector.memset(zbias[:], 0.0)

            # own x block, transposed: [128, itile, 512]
            xTo_t = persist.tile([128, ITILES, NLOC], bf16, tag="xTo")
            nc.sync.dma_start(
                xTo_t[:], xTo.rearrange("(t p) q -> p t q", p=128)
            )

            # weights (shared slots; Wo reuses freed Wq/Wk/Wv slots)
            wq_t = []
            wk_t = []
            wv_t = []
            for it in range(ITILES):
                t = wpool.tile([128, D], bf16, tag="w")
                nc.sync.dma_start(t[:], Wq[it * 128:(it + 1) * 128, :])
                wq_t.append(t)
            for it in range(ITILES):
                t = wpool.tile([128, D], bf16, tag="w")
                nc.sync.dma_start(t[:], Wk[it * 128:(it + 1) * 128, :])
                wk_t.append(t)
            for it in range(ITILES):
                t = wpool.tile([128, D], bf16, tag="w")
                nc.sync.dma_start(t[:], Wv[it * 128:(it + 1) * 128, :])
                wv_t.append(t)

            # K^T per pair: [128 (2 heads x 64 dh), 4096 l]
            kt_t = [
                persist.tile([128, N], bf16, tag=f"kt{p}", name=f"kt{p}")
                for p in range(PAIRS)
            ]
            # V_aug: [128 l-in-tile, 32 ltile, 12 head, 65 (64 v + ones)]
            v_t = persist.tile([128, LTILES, H, DH + 1], bf16, tag="vaug")
            nc.vector.memset(v_t[:, :, :, DH:DH + 1], 1.0)

            # QT: [128 (pair rows), pair, 512 q]
            qt_t = persist.tile([128, PAIRS, NLOC], bf16, tag="qt")

            # attnT pairs: [128 (pair rows), 512 q] bf16 (o-proj rhs)
            attn_t = [
                persist.tile([128, NLOC], bf16, tag=f"attn{p}", name=f"attn{p}")
                for p in range(PAIRS)
            ]

            # ---- QT projection (all pairs up front) ----
            for p in range(PAIRS):
                ps = gp_ps.tile([128, NLOC], f32, tag="gp")
                for it in range(ITILES):
                    nc.tensor.matmul(
                        ps[:],
                        wq_t[it][:, p * 128:(p + 1) * 128],
                        xTo_t[:, it, :],
                        start=(it == 0),
                        stop=(it == ITILES - 1),
                    )
                nc.vector.tensor_scalar_add(
                    qt_t[:, p, :], ps[:], bias_t["bq"][:, p:p + 1]
                )

            # ---- per-group: K/V projection pass + attention ----
            for g in range(GROUPS):
                gpairs = (2 * g, 2 * g + 1)
                # K/V projection for this group, streaming xT
                for ns in range(NSLICES):
                    ch = [None] * ITILES
                    for it in range(ITILES):
                        c = chunks.tile([128, 512], bf16, tag="chunk")
                        nc.sync.dma_start(
                            c[:],
                            xT[it * 128:(it + 1) * 128, ns * 512:(ns + 1) * 512],
                        )
                        ch[it] = c
                    for p in gpairs:
                        ps = gp_ps.tile([128, 512], f32, tag="gp")
                        for it in range(ITILES):
                            nc.tensor.matmul(
                                ps[:],
                                wk_t[it][:, p * 128:(p + 1) * 128],
                                ch[it][:],
                                start=(it == 0),
                                stop=(it == ITILES - 1),
                            )
                        nc.vector.tensor_scalar_add(
                            kt_t[p][:, ns * 512:(ns + 1) * 512],
                            ps[:],
                            bias_t["bk"][:, p:p + 1],
                        )
                    for nsub in range(4):
                        lt = ns * 4 + nsub
                        ps = gp_ps.tile([128, 256], f32, tag="gp")
                        for it in range(ITILES):
                            nc.tensor.matmul(
                                ps[:],
                                ch[it][:, nsub * 128:(nsub + 1) * 128],
                                wv_t[it][:, g * 256:(g + 1) * 256],
                                start=(it == 0),
                                stop=(it == ITILES - 1),
                            )
                        nc.vector.tensor_copy(
                            v_t[:, lt, 4 * g:4 * g + 4, 0:DH],
                            ps[:].rearrange("p (h v) -> p h v", v=DH),
                        )

                # attention for the group's two pairs
                for p in gpairs:
                    accs = []
                    for hh in range(2):
                        accs.append(
                            acc_ps.tile([128, NLOC], f32, tag="acc",
                                        name=f"acc_{p}_{hh}")
                        )
                    for lt in range(LTILES):
                        sc = sc_ps.tile([128, 2, 512], f32, tag="sc")
                        for hh in range(2):
                            nc.tensor.matmul(
                                sc[:, hh, :],
                                kt_t[p][hh * 64:(hh + 1) * 64,
                                        lt * 128:(lt + 1) * 128],
                                qt_t[hh * 64:(hh + 1) * 64, p, :],
                                start=True,
                                stop=True,
                                tile_position=(hh * 64, 0),
                            )
                        ex = expp.tile([128, 2, 512], bf16, tag="exp")
                        nc.scalar.activation(
                            ex[:], sc[:], mybir.ActivationFunctionType.Exp,
                            bias=zbias[:],
                        )
                        for hh in range(2):
                            nc.tensor.matmul(
                                accs[hh][0:DH + 1, :],
                                v_t[:, lt, 2 * p + hh, :],
                                ex[:, hh, :],
                                start=(lt == 0),
                                stop=(lt == LTILES - 1),
                            )
                    for hh in range(2):
                        h = 2 * p + hh
                        acc = accs[hh]
                        # 1/rowsum -> SBUF [1, 512]
                        rs = small.tile([1, NLOC], f32, tag="recip")
                        nc.vector.reciprocal(rs[:], acc[DH:DH + 1, :])
                        rsb = small.tile([1, NLOC], bf16, tag="recipb")
                        nc.vector.tensor_copy(rsb[:], rs[:])
                        # broadcast recip into unused partitions 64:128 of acc
                        nc.tensor.matmul(
                            acc[64:128, :],
                            ones_row[:],
                            rsb[:],
                            start=True,
                            stop=True,
                            tile_position=(0, 64),
                        )
                        bcast_s = small.tile([64, NLOC], f32, tag="bcast")
                        nc.vector.tensor_copy(bcast_s[:], acc[64:128, :])
                        att = attn_t[p][hh * 64:(hh + 1) * 64, :]
                        nc.vector.tensor_tensor(
                            att, acc[0:DH, :], bcast_s[:], mult
                        )
                        nc.vector.tensor_scalar_add(
                            att, att,
                            bias_t["bv"][(h % 2) * 64:(h % 2) * 64 + 64,
                                         h // 2:h // 2 + 1],
                        )

            # ---- output projection (transposed): outT = Wo^T attnT + bo ----
            wo_t = []
            for it in range(ITILES):
                t = wpool.tile([128, D], bf16, tag="w")
                nc.sync.dma_start(t[:], Wo[it * 128:(it + 1) * 128, :])
                wo_t.append(t)
            for ot in range(ITILES):
                ps = gp_ps.tile([128, NLOC], f32, tag="gp")
                for it in range(ITILES):
                    nc.tensor.matmul(
                        ps[:],
                        wo_t[it][:, ot * 128:(ot + 1) * 128],
                        attn_t[it][:],
                        start=(it == 0),
                        stop=(it == ITILES - 1),
                    )
                fo = small.tile([128, NLOC], f32, tag="final")
                nc.vector.tensor_scalar_add(
                    fo[:], ps[:], bias_t["bo"][:, ot:ot + 1]
                )
                nc.sync.dma_start(outT[ot * 128:(ot + 1) * 128, :], fo[:])

    nc.compile()
    return nc


def _prep_inputs(x, Wqkv, bqkv, Wo, bo):
    import ml_dtypes

    bf16 = ml_dtypes.bfloat16
    x2 = np.ascontiguousarray(np.asarray(x, dtype=np.float32).reshape(N, D))
    Wqkv = np.asarray(Wqkv, dtype=np.float32)
    bqkv = np.asarray(bqkv, dtype=np.float32)
    Wo = np.asarray(Wo, dtype=np.float32)
    bo = np.asarray(bo, dtype=np.float32)

    h_idx = np.arange(H).repeat(DH)
    d_idx = np.tile(np.arange(DH), H)
    perm = h_idx * (3 * DH) + d_idx * 3
    s = np.sqrt(np.float32(D))
    Wq = Wqkv[:, perm + 0]
    Wk = Wqkv[:, perm + 1]
    Wv = Wqkv[:, perm + 2] / s
    bq = np.ascontiguousarray(bqkv[perm + 0])
    bk = np.ascontiguousarray(bqkv[perm + 1])
    bv = np.ascontiguousarray(bqkv[perm + 2] / s)

    xT = np.ascontiguousarray(x2.T).astype(bf16)
    shared = {
        "xT": xT,
        "Wq": np.ascontiguousarray(Wq).astype(bf16),
        "Wk": np.ascontiguousarray(Wk).astype(bf16),
        "Wv": np.ascontiguousarray(Wv).astype(bf16),
        "Wo": np.ascontiguousarray(Wo).astype(bf16),
        "bq": bq, "bk": bk, "bv": bv,
        "bo": np.ascontiguousarray(bo),
    }
    in_maps = []
    for c in range(NCORES):
        m = dict(shared)
        m["xTo"] = np.ascontiguousarray(xT[:, c * NLOC:(c + 1) * NLOC])
        in_maps.append(m)
    return in_maps


def kernel(x, Wqkv, bqkv, Wo, bo, _trace=False, _trace_cores=None):
    from concourse.bass_utils import run_bass_kernel_spmd

    if "nc" not in _cache:
        _cache["nc"] = _build_program()
    nc = _cache["nc"]

    in_maps = _prep_inputs(x, Wqkv, bqkv, Wo, bo)
    res = run_bass_kernel_spmd(
        nc, in_maps, list(range(NCORES)), trace=_trace,
        trace_cores=_trace_cores,
    )
    _cache["last_results"] = res
    out = np.concatenate(
        [res.results[c]["outT"].T for c in range(NCORES)], axis=0
    )
    return np.ascontiguousarray(out.reshape(1, N, D).astype(np.float32))
